# revision 11
# baseline (speedup 1.0000x reference)
"""Trainium2 Bass kernel v10 for nn_AttentionLayer.

Math (per core, vocab-sharded): out[b, v'] = occ[b, v'] * leaky_relu(t[v'] + s[b])
with t = table_shard^T a_w (PE, bf16), s = attr_emb @ a_a (DVE, f32).

v8 vs v7 (40.9us). v7 post-mortem: the scalar engine's 5th DMA dispatch
waited ~7us for a HWDGE ring slot and stalled the whole ACT chain queued
behind it; cold-PE matmuls (630ns vs 375ns warm) paced the strips; and
out-dispatches sat between ACTIVATEs. Now:
  - scalar ring carries ONLY aa/attr/awb (3 tiny loads, under the ring
    capacity) then the pure ACT chain; every other DMA lives on sync.
  - sync ring order: per-strip table pairs interleaved with occ chunks,
    then per-(strip,h) stores.
  - PE warm-up: 11 dummy matmuls (no input deps) so HAM is at full clock
    when the real accumulations start; warm-up PSUM shares the pt pool.
  - 4 strips of 1600 instead of 5x1280: fewer ACT/TT ops (less fixed
    overhead), pt = 4 PSUM banks x 2 bufs = all 8 banks.
HBM/core: tbl 3.28 + occ 1.64 (i8; DMA is the saturated critical path,
so fewer bytes beat DVE 2x mode) + out 3.28 = 8.2 MB.
"""

import numpy as np
import ml_dtypes

import concourse.bass as bass
import concourse.tile as tile
from concourse import bacc, mybir
from concourse.bass_utils import run_bass_kernel_spmd

B = 256
L = 512
V = 50257
DW = 256
DA = 256
ALPHA = 0.2

NCORES = 8
VS = 6400          # vocab span per core
SW = 1600          # strip width
NS = VS // SW      # 4 strips
OG = 2 * SW        # occ chunk width (2 strips)

BF16 = ml_dtypes.bfloat16

_CACHE = {}


def _build():
    if "nc" in _CACHE:
        return _CACHE["nc"]
    f32 = mybir.dt.float32
    bf16 = mybir.dt.bfloat16

    nc = bacc.Bacc("TRN2", target_bir_lowering=False, debug=False)
    tbl = nc.declare_dram_parameter("tbl", [128, 2 * VS], bf16, isOutput=False)
    occ = nc.declare_dram_parameter("occ", [128, 2 * VS], mybir.dt.int8, isOutput=False)
    awb = nc.declare_dram_parameter("awb", [128, 2 * 128], bf16, isOutput=False)
    aa = nc.declare_dram_parameter("aa", [128, DA], f32, isOutput=False)
    attr = nc.declare_dram_parameter("attr", [128, 2 * DA], f32, isOutput=False)
    out = nc.declare_dram_parameter("out", [128, 2 * VS], bf16, isOutput=True)

    NCH = ((0, 512), (512, 1024), (1024, 1536), (1536, SW))

    with tile.TileContext(nc) as tc:
        with (
            tc.tile_pool(name="sb", bufs=1) as sb,
            tc.tile_pool(name="pst", bufs=2, space="PSUM") as pst,
        ):
            # ---- tiny loads on the scalar ring (stays under ring capacity,
            # so the ACT chain behind them never stalls on a ring slot) ----
            aa_t = sb.tile([128, DA], f32, tag="aa")
            nc.scalar.dma_start(aa_t[:], aa.ap())
            at = sb.tile([128, 2 * DA], f32, tag="attr")
            nc.scalar.dma_start(at[:], attr.ap())
            awb_t = sb.tile([128, 2 * 128], bf16, tag="awb")
            nc.scalar.dma_start(awb_t[:], awb.ap())

            # ---- PE warm-up: dummy matmuls with no input deps; shares the
            # pt pool so PSUM stays within 8 banks ----
            wres = sb.tile([128, 512], bf16, tag="wres")
            nc.vector.memset(wres[:], 0.0)
            wpt = pst.tile([128, SW], f32, tag="pt", name="wpt")
            for wi in range(11):
                nc.tensor.matmul(
                    wpt[:, 0:512],
                    lhsT=wres[:, 0:128],
                    rhs=wres[:],
                    start=True,
                    stop=True,
                )

            # ---- sync ring: per-strip contiguous table chunks ([dh0|dh1]
            # interleaved by the host) alternating with occ chunks ----
            ts = {}
            ocg = [[None, None], [None, None]]  # [h][grp]
            for si in range(NS):
                t_ = sb.tile([128, 2 * SW], bf16, tag=f"t{si}", name=f"t{si}")
                nc.sync.dma_start(
                    t_[:], tbl.ap()[:, si * 2 * SW : (si + 1) * 2 * SW]
                )
                ts[si] = t_
                if si % 2 == 1:
                    grp = si // 2
                    c0 = grp * OG
                    for h in range(2):
                        o_ = sb.tile([128, OG], mybir.dt.int8, tag=f"oc{h}{grp}", name=f"oc{h}{grp}")
                        nc.sync.dma_start(
                            o_[:], occ.ap()[:, h * VS + c0 : h * VS + c0 + OG]
                        )
                        ocg[h][grp] = o_

            # ---- s = attr_emb @ a_a  (s_sb[:, h] holds b = h*128 + p) ----
            s_sb = sb.tile([128, 2], f32, tag="s")
            for h in range(2):
                pa = sb.tile([128, DA], f32, tag=f"pa{h}")
                nc.vector.tensor_tensor(
                    out=pa[:],
                    in0=at[:, h * DA : (h + 1) * DA],
                    in1=aa_t[:],
                    op=mybir.AluOpType.mult,
                )
                nc.vector.tensor_reduce(
                    out=s_sb[:, h : h + 1],
                    in_=pa[:],
                    axis=mybir.AxisListType.X,
                    op=mybir.AluOpType.add,
                )

            # ---- per strip: matmul + both ACT passes, then mask + store ----
            for si in range(NS):
                grp = si // 2
                off = (si % 2) * SW
                pt = pst.tile([128, SW], f32, tag="pt")
                for dh in range(2):
                    for n0, n1 in NCH:
                        nc.tensor.matmul(
                            pt[:, n0:n1],
                            lhsT=awb_t[:, dh * 128 : (dh + 1) * 128],
                            rhs=ts[si][:, dh * SW + n0 : dh * SW + n1],
                            start=(dh == 0),
                            stop=(dh == 1),
                        )
                for h in range(2):
                    last = si == NS - 1 and h == 1
                    pieces = ((0, 1280), (1280, SW)) if last else ((0, SW),)
                    o1 = sb.tile([128, SW], bf16, tag=f"o1_{si}_{h}", name=f"o1_{si}_{h}")
                    o = sb.tile([128, SW], bf16, tag=f"o_{si}_{h}", name=f"o_{si}_{h}")
                    for p0, p1 in pieces:
                        nc.scalar.activation(
                            o1[:, p0:p1],
                            pt[:, p0:p1],
                            mybir.ActivationFunctionType.Prelu,
                            bias=s_sb[:, h : h + 1],
                            scale=1.0,
                            alpha=ALPHA,
                        )
                        nc.vector.tensor_tensor(
                            out=o[:, p0:p1],
                            in0=o1[:, p0:p1],
                            in1=ocg[h][grp][:, off + p0 : off + p1],
                            op=mybir.AluOpType.mult,
                        )
                        nc.sync.dma_start(
                            out.ap()[:, h * VS + si * SW + p0 : h * VS + si * SW + p1],
                            o[:, p0:p1],
                        )

    nc.compile()
    _CACHE["nc"] = nc
    return nc


def _pmaj(x):
    """[256, N] -> partition-major [128, 2*N] (halves along columns)."""
    n = x.shape[1]
    return np.ascontiguousarray(
        x.reshape(2, 128, n).transpose(1, 0, 2).reshape(128, 2 * n)
    )


def _prep_inputs(words, word_emb_table, attr_emb, a):
    words = np.ascontiguousarray(words).astype(np.int64)
    wet = np.ascontiguousarray(word_emb_table, dtype=np.float32)
    attr = np.ascontiguousarray(attr_emb, dtype=np.float32)
    a = np.ascontiguousarray(a, dtype=np.float32).reshape(-1)

    # awb_dev[p, dh*128 + m] = a[dh*128 + p]
    A = a[:DW].astype(BF16).reshape(2, 128)
    awb_dev = np.ascontiguousarray(
        np.repeat(A.T[:, :, None], 128, axis=2).reshape(128, 2 * 128)
    )
    aa_rep = np.ascontiguousarray(np.broadcast_to(a[DW:][None, :], (128, DA)))
    attr_dev = _pmaj(attr)

    tblpad = np.zeros((NCORES * VS, DW), dtype=np.float32)
    tblpad[:V] = wet
    tbl_bf = tblpad.astype(BF16)

    occ_full = np.zeros((B, NCORES * VS), dtype=np.int8)
    rows = np.repeat(np.arange(B), L)
    occ_full[rows, words.reshape(-1)] = 1

    in_maps = []
    for i in range(NCORES):
        blk = tbl_bf[i * VS : (i + 1) * VS, :]          # [VS, 256]
        # [128, NS*2*SW]: per strip, dh0 then dh1 columns (contiguous chunk)
        tbl_dev = np.ascontiguousarray(
            blk.T.reshape(2, 128, NS, SW)
            .transpose(1, 2, 0, 3)
            .reshape(128, 2 * VS)
        )
        occ_dev = _pmaj(occ_full[:, i * VS : (i + 1) * VS])
        in_maps.append(
            {
                "tbl": tbl_dev,
                "occ": occ_dev,
                "awb": awb_dev,
                "aa": aa_rep,
                "attr": attr_dev,
            }
        )
    return in_maps


def kernel(words, word_emb_table, attr_emb, a, _trace=False, **_kw):
    nc = _build()
    in_maps = _prep_inputs(words, word_emb_table, attr_emb, a)
    res = run_bass_kernel_spmd(nc, in_maps, list(range(NCORES)), trace=_trace)
    parts = []
    for i in range(NCORES):
        o = res.results[i]["out"]                       # [128, 2*VS] bf16
        parts.append(o.reshape(128, 2, VS).transpose(1, 0, 2).reshape(B, VS))
    out = np.ascontiguousarray(
        np.concatenate(parts, axis=1)[:, :V].astype(np.float32)
    )
    if _trace:
        return out, res
    return out


# revision 12
# speedup vs baseline: 1.0485x; 1.0485x over previous
"""Trainium2 Bass kernel v11 for nn_AttentionLayer.

Math (per core, vocab-sharded): out[b, v'] = occ[b, v'] * leaky_relu(t[v'] + s[b])
with t = table_shard^T a_w (PE, bf16), s = attr_emb @ a_a (DVE).

v11 vs v9/v10 (~39.5-41.7us). Measured structure: exec ~= first_ACT +
ACT-chain + tail; DMA stream saturates at ~0.4 MB/us and is not the
binding constraint once bytes are ~8.2 MB. So:
  - strip 0 is only 512 wide: its table chunk (262 KB) lands ~11.5us and
    two cold matmuls later the ACT chain starts ~13 (was 17.6-19.5).
  - remaining 4 strips of 1472 (3 PSUM banks x 2 bufs + 1 bank for
    strip 0 = 7 of 8 banks).
  - no PE warm-up (never flipped HAM in time), no split tail (v10 showed
    both regress).
  - aa/attr in bf16 - lighter and lands earlier for the s bias.
HBM/core: tbl 3.28 + occ 1.64 (i8) + out 3.28 + 0.2 smalls = 8.4 MB.
"""

import numpy as np
import ml_dtypes

import concourse.bass as bass
import concourse.tile as tile
from concourse import bacc, mybir
from concourse.bass_utils import run_bass_kernel_spmd

B = 256
L = 512
V = 50257
DW = 256
DA = 256
ALPHA = 0.2

NCORES = 8
VS = 6400
WIDTHS = (512, 1472, 1472, 1472, 1472)   # strip widths, sum = VS
NS = len(WIDTHS)
OFFS = tuple(np.cumsum((0,) + WIDTHS).tolist())  # col offsets, len NS+1
OCC_SPLIT = 3456                          # occ chunk boundary = OFFS[3]

BF16 = ml_dtypes.bfloat16

_CACHE = {}


def _nchunks(w):
    c, n0 = [], 0
    while n0 < w:
        n1 = min(n0 + 512, w)
        c.append((n0, n1))
        n0 = n1
    return tuple(c)


def _build():
    if "nc" in _CACHE:
        return _CACHE["nc"]
    f32 = mybir.dt.float32
    bf16 = mybir.dt.bfloat16
    i8 = mybir.dt.int8

    nc = bacc.Bacc("TRN2", target_bir_lowering=False, debug=False)
    tbl = nc.declare_dram_parameter("tbl", [128, 2 * VS], bf16, isOutput=False)
    occ = nc.declare_dram_parameter("occ", [128, 2 * VS], i8, isOutput=False)
    awb = nc.declare_dram_parameter("awb", [128, 2 * 128], bf16, isOutput=False)
    aa = nc.declare_dram_parameter("aa", [128, DA], bf16, isOutput=False)
    attr = nc.declare_dram_parameter("attr", [128, 2 * DA], bf16, isOutput=False)
    out = nc.declare_dram_parameter("out", [128, 2 * VS], bf16, isOutput=True)

    with tile.TileContext(nc) as tc:
        with (
            tc.tile_pool(name="sb", bufs=1) as sb,
            tc.tile_pool(name="ps0", bufs=1, space="PSUM") as ps0,
            tc.tile_pool(name="psm", bufs=2, space="PSUM") as psm,
        ):
            # ---- tiny loads on the scalar ring ----
            aa_t = sb.tile([128, DA], bf16, tag="aa")
            nc.scalar.dma_start(aa_t[:], aa.ap())
            at = sb.tile([128, 2 * DA], bf16, tag="attr")
            nc.scalar.dma_start(at[:], attr.ap())
            awb_t = sb.tile([128, 2 * 128], bf16, tag="awb")
            nc.scalar.dma_start(awb_t[:], awb.ap())

            # ---- sync ring: per-strip table chunks ([dh0|dh1] interleaved
            # by host) with occ chunks slotted between ----
            ts = {}
            ocg = [[None, None], [None, None]]
            occ_bounds = ((0, OCC_SPLIT), (OCC_SPLIT, VS))
            for si in range(NS):
                w = WIDTHS[si]
                t_ = sb.tile([128, 2 * w], bf16, tag=f"t{si}", name=f"t{si}")
                nc.sync.dma_start(
                    t_[:], tbl.ap()[:, 2 * OFFS[si] : 2 * OFFS[si + 1]]
                )
                ts[si] = t_
                if si == 2:
                    for h in range(2):
                        c0, c1 = occ_bounds[0]
                        o_ = sb.tile([128, c1 - c0], i8, tag=f"oc{h}0", name=f"oc{h}0")
                        nc.sync.dma_start(
                            o_[:], occ.ap()[:, h * VS + c0 : h * VS + c1]
                        )
                        ocg[h][0] = o_
                if si == 4:
                    for h in range(2):
                        c0, c1 = occ_bounds[1]
                        o_ = sb.tile([128, c1 - c0], i8, tag=f"oc{h}1", name=f"oc{h}1")
                        nc.sync.dma_start(
                            o_[:], occ.ap()[:, h * VS + c0 : h * VS + c1]
                        )
                        ocg[h][1] = o_

            # ---- s = attr_emb @ a_a ----
            s_sb = sb.tile([128, 2], f32, tag="s")
            for h in range(2):
                pa = sb.tile([128, DA], f32, tag=f"pa{h}")
                nc.vector.tensor_tensor(
                    out=pa[:],
                    in0=at[:, h * DA : (h + 1) * DA],
                    in1=aa_t[:],
                    op=mybir.AluOpType.mult,
                )
                nc.vector.tensor_reduce(
                    out=s_sb[:, h : h + 1],
                    in_=pa[:],
                    axis=mybir.AxisListType.X,
                    op=mybir.AluOpType.add,
                )

            # ---- per strip: matmul, ACT x2, mask x2, store x2 ----
            for si in range(NS):
                w = WIDTHS[si]
                grp = 0 if OFFS[si] < OCC_SPLIT else 1
                off = OFFS[si] - (0 if grp == 0 else OCC_SPLIT)
                pool = ps0 if si == 0 else psm
                pt = pool.tile([128, w], f32, tag="pt0" if si == 0 else "pt",
                               name=f"pt{si}")
                for dh in range(2):
                    for n0, n1 in _nchunks(w):
                        nc.tensor.matmul(
                            pt[:, n0:n1],
                            lhsT=awb_t[:, dh * 128 : (dh + 1) * 128],
                            rhs=ts[si][:, dh * w + n0 : dh * w + n1],
                            start=(dh == 0),
                            stop=(dh == 1),
                        )
                for h in range(2):
                    o1 = sb.tile([128, w], bf16, tag=f"o1_{si}_{h}", name=f"o1_{si}_{h}")
                    nc.scalar.activation(
                        o1[:],
                        pt[:],
                        mybir.ActivationFunctionType.Prelu,
                        bias=s_sb[:, h : h + 1],
                        scale=1.0,
                        alpha=ALPHA,
                    )
                    o = sb.tile([128, w], bf16, tag=f"o_{si}_{h}", name=f"o_{si}_{h}")
                    nc.vector.tensor_tensor(
                        out=o[:],
                        in0=o1[:],
                        in1=ocg[h][grp][:, off : off + w],
                        op=mybir.AluOpType.mult,
                    )
                    nc.sync.dma_start(
                        out.ap()[:, h * VS + OFFS[si] : h * VS + OFFS[si + 1]],
                        o[:],
                    )

    nc.compile()
    _CACHE["nc"] = nc
    return nc


def _pmaj(x):
    """[256, N] -> partition-major [128, 2*N] (halves along columns)."""
    n = x.shape[1]
    return np.ascontiguousarray(
        x.reshape(2, 128, n).transpose(1, 0, 2).reshape(128, 2 * n)
    )


def _prep_inputs(words, word_emb_table, attr_emb, a):
    words = np.ascontiguousarray(words).astype(np.int64)
    wet = np.ascontiguousarray(word_emb_table, dtype=np.float32)
    attr = np.ascontiguousarray(attr_emb, dtype=np.float32)
    a = np.ascontiguousarray(a, dtype=np.float32).reshape(-1)

    A = a[:DW].astype(BF16).reshape(2, 128)
    awb_dev = np.ascontiguousarray(
        np.repeat(A.T[:, :, None], 128, axis=2).reshape(128, 2 * 128)
    )
    aa_rep = np.ascontiguousarray(
        np.broadcast_to(a[DW:].astype(BF16)[None, :], (128, DA))
    )
    attr_dev = _pmaj(attr.astype(BF16))

    tblpad = np.zeros((NCORES * VS, DW), dtype=np.float32)
    tblpad[:V] = wet
    tbl_bf = tblpad.astype(BF16)

    occ_full = np.zeros((B, NCORES * VS), dtype=np.int8)
    rows = np.repeat(np.arange(B), L)
    occ_full[rows, words.reshape(-1)] = 1

    in_maps = []
    for i in range(NCORES):
        blk = tbl_bf[i * VS : (i + 1) * VS, :]          # [VS, 256]
        bt = blk.T.reshape(2, 128, VS)                  # [dh, p, v]
        # per-strip contiguous [dh0 cols | dh1 cols] chunks
        cols = []
        for si in range(NS):
            seg = bt[:, :, OFFS[si] : OFFS[si + 1]]     # [2, 128, w]
            cols.append(seg.transpose(1, 0, 2).reshape(128, -1))
        tbl_dev = np.ascontiguousarray(np.concatenate(cols, axis=1))
        occ_dev = _pmaj(occ_full[:, i * VS : (i + 1) * VS])
        in_maps.append(
            {
                "tbl": tbl_dev,
                "occ": occ_dev,
                "awb": awb_dev,
                "aa": aa_rep,
                "attr": attr_dev,
            }
        )
    return in_maps


def kernel(words, word_emb_table, attr_emb, a, _trace=False, **_kw):
    nc = _build()
    in_maps = _prep_inputs(words, word_emb_table, attr_emb, a)
    res = run_bass_kernel_spmd(nc, in_maps, list(range(NCORES)), trace=_trace)
    parts = []
    for i in range(NCORES):
        o = res.results[i]["out"]                       # [128, 2*VS] bf16
        parts.append(o.reshape(128, 2, VS).transpose(1, 0, 2).reshape(B, VS))
    out = np.ascontiguousarray(
        np.concatenate(parts, axis=1)[:, :V].astype(np.float32)
    )
    if _trace:
        return out, res
    return out
